# revision 1
# baseline (speedup 1.0000x reference)
"""CrossAttentionFusion TRN2 kernel: 8-core data-parallel Bass/Tile implementation.

Per core (B_loc = 2048):
  pass A: V = feat2 @ Wv           (form B, stationary = X2^T blocks)  -> v_dram
  pass B: Q^T = Wq^T-chunks @ X1^T (form A), K^T likewise
          scores_T[g,h] per sample via packed 8x8 PE matmuls
          softmax over g (exp via ACT, sums via strip-mask matmul, DVE recip)
          ctx[h,d] per sample via packed K=8 matmuls (M=32 junk padding)
          ctx -> ctx^T via PE transposes -> ctxT_dram  (float32r)
  pass C: out = ctx @ Wo           (form B, stationary = ctxT blocks)

Big matmuls run in float32r (TF32-class, ~1.5e-4 rel err); the attention
middle runs in plain fp32 (tiny-N matmuls are the same speed in fp32).
"""

import sys

sys.path.insert(0, "/opt/trn_rl_repo")

import numpy as np
import concourse.bacc as bacc
import concourse.mybir as mybir
import concourse.tile as tile
from concourse.masks import make_identity

B = 16384
DIM = 1024
H = 8
HD = 128
SCALE = float(np.sqrt(HD))
NCORES = 8
B_LOC = B // NCORES  # 2048
SLAB = 512
NSLAB = B_LOC // SLAB  # 4
SUB = 128
NSUB = SLAB // SUB  # 4

F32 = mybir.dt.float32
F32R = mybir.dt.float32r

_nc_cache = {}
TIME_LOOP_N = None  # when set, wraps the whole compute in a HW For_i loop (timing)


def build_nc():
    import concourse.bass as bass

    nc = bacc.Bacc(None)
    feat1 = nc.declare_dram_parameter("feat1", [B_LOC, DIM], F32, isOutput=False)
    feat2 = nc.declare_dram_parameter("feat2", [B_LOC, DIM], F32, isOutput=False)
    Wq = nc.declare_dram_parameter("Wq", [DIM, DIM], F32, isOutput=False)
    Wk = nc.declare_dram_parameter("Wk", [DIM, DIM], F32, isOutput=False)
    Wv = nc.declare_dram_parameter("Wv", [DIM, DIM], F32, isOutput=False)
    Wo = nc.declare_dram_parameter("Wo", [DIM, DIM], F32, isOutput=False)
    out = nc.declare_dram_parameter("out", [B_LOC, DIM], F32, isOutput=True)

    v_dram = nc.dram_tensor("v_dram", [B_LOC, DIM], F32)
    ctxt_dram = nc.dram_tensor("ctxt_dram", [HD, H, B_LOC], F32R)

    w_view = lambda W: W.rearrange("(c p) n -> p c n", p=128)  # (f-part, fchunk, o)

    with tile.TileContext(nc) as tc:
        with (
            tc.tile_pool(name="const", bufs=1) as cpool,
            tc.tile_pool(name="w", bufs=1) as wpool,
            tc.tile_pool(name="feat", bufs=1) as fpool,
            tc.tile_pool(name="xt", bufs=1) as xtpool,
            tc.tile_pool(name="qk", bufs=1) as qkpool,
            tc.tile_pool(name="small", bufs=1) as spool,
            tc.tile_pool(name="vresh", bufs=1) as vpool,
            tc.tile_pool(name="csb", bufs=1) as cpool2,
            tc.tile_pool(name="ctxt", bufs=1) as ctpool,
            tc.tile_pool(name="p2", bufs=1) as p2pool,
            tc.tile_pool(name="ps_work", bufs=2, space="PSUM") as ps_work,
            tc.tile_pool(name="ps_scsum", bufs=1, space="PSUM") as ps_scsum,
            tc.tile_pool(name="ps_ctx", bufs=1, space="PSUM") as ps_ctxp,
            tc.tile_pool(name="ps_ctxt", bufs=1, space="PSUM") as ps_ctxtp,
        ):
            ident = cpool.tile([128, 128], F32, tag="ident")
            make_identity(nc, ident)
            # strip mask: SM[k, m] = 1 if k//32 == m//32 and k%32 < 8
            smask = cpool.tile([128, 128], F32, tag="smask")
            nc.gpsimd.memset(smask[:], 0.0)
            sm4 = smask[:].rearrange("(j r) m -> j r m", j=4)
            for j in range(4):
                nc.gpsimd.memset(sm4[j, 0:8, 32 * j:32 * (j + 1)], 1.0)

            def transpose_in(feat_sb, xt_tile):
                """feat_sb [128, 1024] fp32 -> xt_tile [128, 8, 128] slices (fp32r),
                via 2 psum tiles of 4 blocks each. Returns nothing."""
                for t in range(2):
                    pt = ps_work.tile([128, 512], F32, tag="work")
                    for j in range(4):
                        c = 4 * t + j
                        nc.tensor.transpose(
                            pt[:, j * 128:(j + 1) * 128],
                            feat_sb[:, c * 128:(c + 1) * 128],
                            ident[:],
                        )
                    nc.vector.tensor_copy(
                        xt_tile[:, 4 * t:4 * t + 4, :], pt[:].rearrange("p (j b) -> p j b", j=4)
                    )

            def emit_all():
                # ---------------- pass A: V ----------------
                wv = wpool.tile([128, 8, DIM], F32R, tag="wv")
                nc.gpsimd.dma_start(out=wv[:], in_=w_view(Wv))
                for sl in range(NSLAB):
                    for bt in range(NSUB):
                        b0 = sl * SLAB + bt * SUB
                        f2 = fpool.tile([128, DIM], F32, tag="f1")
                        nc.sync.dma_start(out=f2[:], in_=feat2[b0:b0 + 128, :])
                        x2t = xtpool.tile([128, 8, 128], F32R, tag="x2ta")
                        transpose_in(f2, x2t)
                        for half in range(2):
                            pv = ps_work.tile([128, 512], F32, tag="work")
                            for fc in range(8):
                                nc.tensor.matmul(
                                    pv[:],
                                    x2t[:, fc, :],
                                    wv[:, fc, half * 512:(half + 1) * 512],
                                    start=(fc == 0),
                                    stop=(fc == 7),
                                )
                            vsb = cpool2.tile([128, 512], F32, tag="vsb")
                            nc.scalar.copy(vsb[:], pv[:])
                            nc.sync.dma_start(
                                out=v_dram[b0:b0 + 128, half * 512:(half + 1) * 512],
                                in_=vsb[:],
                            )

                # ---------------- pass B ----------------
                wq = wpool.tile([128, 8, DIM], F32R, tag="wq")
                wk = wpool.tile([128, 8, DIM], F32R, tag="wk")
                nc.gpsimd.dma_start(out=wq[:], in_=w_view(Wq))
                nc.gpsimd.dma_start(out=wk[:], in_=w_view(Wk))

                for sl in range(NSLAB):
                    # transpose inputs for this slab
                    x1t = xtpool.tile([128, 8, SLAB], F32R, tag="x1t")
                    x2t = xtpool.tile([128, 8, SLAB], F32R, tag="x2t")
                    for bt in range(NSUB):
                        b0 = sl * SLAB + bt * SUB
                        f1 = fpool.tile([128, DIM], F32, tag="f1")
                        f2 = fpool.tile([128, DIM], F32, tag="f2")
                        nc.sync.dma_start(out=f1[:], in_=feat1[b0:b0 + 128, :])
                        nc.sync.dma_start(out=f2[:], in_=feat2[b0:b0 + 128, :])
                        transpose_in(f1, x1t[:, :, bt * 128:(bt + 1) * 128])
                        transpose_in(f2, x2t[:, :, bt * 128:(bt + 1) * 128])
                    # Q^T, K^T (form A): psum[oc-part, b] = sum_fc W[:, fc, oc]^T-ish
                    qt = qkpool.tile([128, 8, SLAB], F32, tag="qt")
                    kt = qkpool.tile([128, 8, SLAB], F32, tag="kt")
                    for (wmat, dst) in ((wq, qt), (wk, kt)):
                        for oc in range(8):
                            pq = ps_work.tile([128, 512], F32, tag="work")
                            for fc in range(8):
                                nc.tensor.matmul(
                                    pq[:],
                                    wmat[:, fc, oc * 128:(oc + 1) * 128],
                                    x1t[:, fc, :] if dst is qt else x2t[:, fc, :],
                                    start=(fc == 0),
                                    stop=(fc == 7),
                                )
                            nc.scalar.copy(dst[:, oc, :], pq[:])

                    for sub in range(NSUB):
                        b0 = sl * SLAB + sub * SUB
                        # scores: sample s: strip i = s%4, col m = s//4
                        psc = ps_scsum.tile([128, 256], F32, tag="scsum")
                        nc.vector.memset(psc[:], 0.0)
                        for s in range(SUB):
                            i, m = s % 4, s // 4
                            loc = sub * SUB + s
                            nc.tensor.matmul(
                                psc[32 * i:32 * i + 8, m * 8:m * 8 + 8],
                                kt[:, :, loc],
                                qt[:, :, loc],
                                start=True, stop=True,
                                tile_position=(0, 32 * i),
                            )
                        e_sb = spool.tile([128, 256], F32, tag="esb")
                        nc.scalar.activation(
                            e_sb[:], psc[:], mybir.ActivationFunctionType.Exp,
                            bias=0.0, scale=float(1.0 / SCALE),
                        )
                        psum_s = ps_scsum.tile([128, 256], F32, tag="scsum")
                        nc.tensor.matmul(psum_s[:], smask[:], e_sb[:], start=True, stop=True)
                        r_sb = spool.tile([128, 256], F32, tag="rsb")
                        nc.vector.reciprocal(r_sb[:], psum_s[:])
                        a_sb = spool.tile([128, 288], F32, tag="asb")
                        nc.vector.memset(a_sb[:, 256:288], 0.0)
                        nc.vector.tensor_mul(a_sb[:, 0:256], e_sb[:], r_sb[:])

                        # V reshape from DRAM: v_resh[32i+g, m*128+d] = V[b0+4m+i, g*128+d]
                        v_resh = vpool.tile([128, 4096], F32, tag="vresh")
                        vr4 = v_resh[:].rearrange("(i r) (m d) -> i r m d", i=4, d=128)
                        vsrc = v_dram[b0:b0 + 128, :].rearrange("(m i) (g d) -> i g m d", i=4, d=128)
                        for i in range(4):
                            nc.sync.dma_start(out=vr4[i, 0:8], in_=vsrc[i])

                        ctxt_sb = ctpool.tile([128, 1024], F32R, tag="ctxt")
                        for uh in range(2):
                            ps_i = [ps_ctxp.tile([128, 512], F32, tag=f"ctx{i}", name=f"psctx{i}") for i in range(4)]
                            for t in range(64):
                                s = 64 * uh + t
                                i, m = s % 4, s // 4
                                jo, u = m % 4, m // 4
                                uu = u % 4
                                nc.tensor.matmul(
                                    ps_i[i][32 * jo:32 * jo + 32, uu * 128:(uu + 1) * 128],
                                    a_sb[32 * i:32 * i + 8, m * 8:m * 8 + 32],
                                    v_resh[32 * i:32 * i + 8, m * 128:(m + 1) * 128],
                                    start=True, stop=True,
                                    tile_position=(32 * i, 32 * jo),
                                )
                            for i in range(4):
                                c_sb = cpool2.tile([128, 512], F32, tag="csb", bufs=2)
                                nc.scalar.copy(c_sb[:], ps_i[i][:])
                                pct = ps_ctxtp.tile([128, 512], F32, tag="ctxt_ps")
                                for uu in range(4):
                                    nc.tensor.transpose(
                                        pct[:, uu * 128:(uu + 1) * 128],
                                        c_sb[:, uu * 128:(uu + 1) * 128],
                                        ident[:],
                                    )
                                # scatter: ctxt_sb[d, h*128 + 16*(4uh+uu) + 4jo + i]
                                #   <- pct[d, uu*128 + 32jo + h]
                                sct = pct[:].rearrange("p (uu jo r) -> p uu jo r", uu=4, jo=4)[:, :, :, 0:8]
                                nc.vector.tensor_copy(
                                    ctxt_sb[:].rearrange(
                                        "p (h u w e) -> p u w h e", u=8, w=4, e=4
                                    )[:, 4 * uh:4 * uh + 4, :, :, i],
                                    sct,
                                )
                        nc.sync.dma_start(
                            out=ctxt_dram[:, :, b0:b0 + 128],
                            in_=ctxt_sb[:].rearrange("d (h b) -> d h b", h=8),
                        )

                # ---------------- pass C: out = ctx @ Wo ----------------
                wo = wpool.tile([128, 8, DIM], F32R, tag="wv")
                nc.gpsimd.dma_start(out=wo[:], in_=w_view(Wo))
                for j in range(B_LOC // 128):
                    ct = p2pool.tile([128, 8, 128], F32R, tag="ct")
                    nc.sync.dma_start(
                        out=ct[:],
                        in_=ctxt_dram[:, :, j * 128:(j + 1) * 128],
                    )
                    for half in range(2):
                        po = ps_work.tile([128, 512], F32, tag="work")
                        for hc in range(8):
                            nc.tensor.matmul(
                                po[:],
                                ct[:, hc, :],
                                wo[:, hc, half * 512:(half + 1) * 512],
                                start=(hc == 0),
                                stop=(hc == 7),
                            )
                        osb = cpool2.tile([128, 512], F32, tag="vsb")
                        nc.scalar.copy(osb[:], po[:])
                        nc.sync.dma_start(
                            out=out[j * 128:(j + 1) * 128, half * 512:(half + 1) * 512],
                            in_=osb[:],
                        )

            if TIME_LOOP_N:
                with tc.For_i(0, TIME_LOOP_N, 1) as _iv:
                    emit_all()
            else:
                emit_all()
    nc.compile()
    return nc


def _numpy_fallback(feat1, feat2, Wq, bq, Wk, bk, Wv, bv, Wo, bo):
    def sm(x):
        x = x - x.max(-1, keepdims=True)
        e = np.exp(x)
        return e / e.sum(-1, keepdims=True)

    b = feat1.shape[0]
    Q = (feat1 @ Wq + bq).reshape(b, H, HD)
    K = (feat2 @ Wk + bk).reshape(b, H, HD)
    V = (feat2 @ Wv + bv).reshape(b, H, HD)
    s = np.einsum("bhd,bgd->bhg", Q, K) / SCALE
    a = sm(s)
    ctx = np.einsum("bhg,bgd->bhd", a, V).reshape(b, DIM)
    return (ctx @ Wo + bo).astype(np.float32)


def kernel(feat1, feat2, Wq, bq, Wk, bk, Wv, bv, Wo, bo):
    feat1 = np.ascontiguousarray(np.asarray(feat1, dtype=np.float32))
    feat2 = np.ascontiguousarray(np.asarray(feat2, dtype=np.float32))
    Wq = np.ascontiguousarray(np.asarray(Wq, dtype=np.float32))
    Wk = np.ascontiguousarray(np.asarray(Wk, dtype=np.float32))
    Wv = np.ascontiguousarray(np.asarray(Wv, dtype=np.float32))
    Wo = np.ascontiguousarray(np.asarray(Wo, dtype=np.float32))
    bq, bk, bv, bo = (np.asarray(x, dtype=np.float32) for x in (bq, bk, bv, bo))
    if any(np.abs(x).max() > 0 for x in (bq, bk, bv, bo) if x.size):
        return _numpy_fallback(feat1, feat2, Wq, bq, Wk, bk, Wv, bv, Wo, bo)

    from concourse.bass_utils import run_bass_kernel_spmd

    if "nc" not in _nc_cache:
        _nc_cache["nc"] = build_nc()
    nc = _nc_cache["nc"]

    in_maps = []
    for c in range(NCORES):
        sl = slice(c * B_LOC, (c + 1) * B_LOC)
        in_maps.append({
            "feat1": feat1[sl], "feat2": feat2[sl],
            "Wq": Wq, "Wk": Wk, "Wv": Wv, "Wo": Wo,
        })
    res = run_bass_kernel_spmd(nc, in_maps, list(range(NCORES)))
    return np.concatenate([res.results[c]["out"] for c in range(NCORES)], axis=0)



# revision 3
# speedup vs baseline: 11.9256x; 11.9256x over previous
"""CrossAttentionFusion TRN2 kernel v2: fused single-pass bf16 pipeline, 8-core data parallel.

Per core (B_loc = 2048), per slab of 512 samples:
  - DMA feat blocks, DVE-convert fp32->bf16, PE-transpose -> x1t/x2t [f, b] bf16
  - Q^T/K^T/V^T via form-A matmuls (stationary = W bf16 chunks): qt/kt/vt [d, h, b]
  - middle per 16-sample tile: ONE packed 128x128 scores matmul (cols (s,h), rows (s,g)),
    exp via ACT, block-diag mask via DVE, sums via blockdiag-ones matmul, recip DVE,
    attn = e_m * r, V^T tile -> V_pack via PE transpose, ctx^T = V_pack^T-matmul(attn)
  - out = ctx @ Wo via form-B (stationary = ctx^T chunks straight from SBUF)
No DRAM intermediates; everything bf16 except PSUM accumulation + softmax stats.
"""

import sys

sys.path.insert(0, "/opt/trn_rl_repo")

import numpy as np
import concourse.bacc as bacc
import concourse.mybir as mybir
import concourse.tile as tile
from concourse.masks import make_identity, make_block_diagonal

B = 16384
DIM = 1024
H = 8
HD = 128
SCALE = float(np.sqrt(HD))
NCORES = 8
B_LOC = B // NCORES  # 2048
SLAB = 512
NSLAB = B_LOC // SLAB  # 4
SUB = 128
NSUB = SLAB // SUB  # 4
TS = 16  # samples per packed tile
TPS = SUB // TS  # 8 tiles per sub

F32 = mybir.dt.float32
BF16 = mybir.dt.bfloat16

_nc_cache = {}
TIME_LOOP_N = None


def build_nc():
    nc = bacc.Bacc(None)
    feat1 = nc.declare_dram_parameter("feat1", [B_LOC, DIM], F32, isOutput=False)
    feat2 = nc.declare_dram_parameter("feat2", [B_LOC, DIM], F32, isOutput=False)
    Wq = nc.declare_dram_parameter("Wq", [DIM, DIM], F32, isOutput=False)
    Wk = nc.declare_dram_parameter("Wk", [DIM, DIM], F32, isOutput=False)
    Wv = nc.declare_dram_parameter("Wv", [DIM, DIM], F32, isOutput=False)
    Wo = nc.declare_dram_parameter("Wo", [DIM, DIM], F32, isOutput=False)
    out = nc.declare_dram_parameter("out", [B_LOC, DIM], F32, isOutput=True)

    w_view = lambda W: W.rearrange("(c p) n -> p c n", p=128)  # f=(c,p), out n

    with tile.TileContext(nc) as tc:
        with (
            tc.tile_pool(name="const", bufs=1) as cpool,
            tc.tile_pool(name="wbf", bufs=1) as wpool,
            tc.tile_pool(name="wstage", bufs=2) as wstg,
            tc.tile_pool(name="fstage", bufs=4) as fstg,
            tc.tile_pool(name="fbf", bufs=4) as fbfp,
            tc.tile_pool(name="xt", bufs=1) as xtp,
            tc.tile_pool(name="qkv", bufs=1) as qkvp,
            tc.tile_pool(name="esb", bufs=2) as ep,
            tc.tile_pool(name="em", bufs=2) as emp,
            tc.tile_pool(name="rsb", bufs=2) as rp,
            tc.tile_pool(name="attn", bufs=2) as ap,
            tc.tile_pool(name="vpk", bufs=2) as vp,
            tc.tile_pool(name="ctxt", bufs=2) as ctp,
            tc.tile_pool(name="outsb", bufs=4) as op,
            tc.tile_pool(name="ps_work", bufs=2, space="PSUM") as ps_work,
            tc.tile_pool(name="ps_sc", bufs=2, space="PSUM") as ps_sc,
            tc.tile_pool(name="ps_mid", bufs=4, space="PSUM") as ps_mid,
        ):
            ident = cpool.tile([128, 128], BF16, tag="ident")
            make_identity(nc, ident)
            # m01[128, 1024]: block t (128 cols) has blockdiag8 ones pattern:
            # m01[8s+g, t*128 + 8s'+h] = (s == s')
            m01 = cpool.tile([128, 1024], BF16, tag="m01")
            make_block_diagonal(nc, m01[:, 0:128], 8)
            for t in range(1, 8):
                nc.vector.tensor_copy(m01[:, t * 128:(t + 1) * 128], m01[:, 0:128])
            ones128 = cpool.tile([128, 128], BF16, tag="ones128")
            nc.gpsimd.memset(ones128[:], 1.0)

            def load_weights():
                """DMA fp32 weights, convert to bf16 in SBUF. Returns dict."""
                wbf = {}
                for i, (name, W) in enumerate((("wq", Wq), ("wk", Wk), ("wv", Wv), ("wo", Wo))):
                    wbf[name] = wpool.tile([128, 8, 1024], BF16, tag=name, name=name)
                for i, (name, W) in enumerate((("wq", Wq), ("wk", Wk), ("wv", Wv), ("wo", Wo))):
                    for j in range(4):  # 2 chunks per step
                        stg = wstg.tile([128, 2, 1024], F32, tag="wstage")
                        nc.gpsimd.dma_start(out=stg[:], in_=w_view(W)[:, 2 * j:2 * j + 2, :])
                        if (i * 4 + j) % 2 == 0:
                            nc.vector.tensor_copy(wbf[name][:, 2 * j:2 * j + 2, :], stg[:])
                        else:
                            nc.scalar.copy(wbf[name][:, 2 * j:2 * j + 2, :], stg[:])
                return wbf

            def transpose_in(fb, dst, blk):
                """fb [128,1024] bf16 -> dst[:, :, blk*128:(blk+1)*128] (dst [128,8,512])."""
                for t in range(2):
                    pt = ps_work.tile([128, 512], BF16, tag="work")
                    for j in range(4):
                        c = 4 * t + j
                        nc.tensor.transpose(
                            pt[:, j * 128:(j + 1) * 128],
                            fb[:, c * 128:(c + 1) * 128],
                            ident[:],
                        )
                    nc.scalar.copy(
                        dst[:, 4 * t:4 * t + 4, blk * 128:(blk + 1) * 128],
                        pt[:].rearrange("p (j b) -> p j b", j=4),
                    )

            def emit_all():
                wbf = load_weights()

                # pipeline state: pending sub whose ctx+O aren't emitted yet
                pending = []

                def emit_stage2(st):
                    """ctx matmuls (unnormalized) + normalize + O-projection for a finished sub."""
                    sl, sub, e_m, r_sb, vpk_sb, ctxt_sb = st
                    pc1 = ps_mid.tile([128, 512], F32, tag="mid")
                    pc2 = ps_mid.tile([128, 512], F32, tag="mid")
                    for t in range(TPS):
                        pc = pc1 if t < 4 else pc2
                        nc.tensor.matmul(
                            pc[:, (t % 4) * 128:(t % 4 + 1) * 128],
                            vpk_sb[:, t * 128:(t + 1) * 128],
                            e_m[:, t * 128:(t + 1) * 128],
                            start=True, stop=True,
                        )
                    # ctxt col = h*128 + t*16 + s so O-proj stationary is contiguous
                    ctxt_v = ctxt_sb[:].rearrange("d (h t s) -> d t h s", h=8, t=8, s=16)
                    with nc.allow_low_precision(reason="softmax normalize in bf16"):
                        for pi, pc in ((0, pc1), (1, pc2)):
                            nc.vector.tensor_mul(
                                ctxt_v[:, 4 * pi:4 * pi + 4, :, :],
                                pc[:].rearrange("p (t s h) -> p t h s", t=4, s=16, h=8),
                                r_sb[:, 512 * pi:512 * (pi + 1)].rearrange(
                                    "p (t s h) -> p t h s", t=4, s=16, h=8),
                            )
                    # O-projection for this sub: out rows = b = sub block
                    b0 = sl * SLAB + sub * SUB
                    for half in range(2):
                        po = ps_work.tile([128, 512], F32, tag="work")
                        for h in range(8):
                            nc.tensor.matmul(
                                po[:],
                                ctxt_sb[:, h * 128:(h + 1) * 128],
                                wbf["wo"][:, h, half * 512:(half + 1) * 512],
                                start=(h == 0), stop=(h == 7),
                            )
                        osb = op.tile([128, 512], F32, tag="osb")
                        if half == 0:
                            nc.vector.tensor_copy(osb[:], po[:])
                        else:
                            nc.scalar.copy(osb[:], po[:])
                        nc.sync.dma_start(
                            out=out[b0:b0 + 128, half * 512:(half + 1) * 512],
                            in_=osb[:],
                        )

                def stage_feat(sl):
                    """DMA + bf16-convert one slab's feat blocks; returns list of bf16 tiles."""
                    tiles = []
                    for blk in range(NSUB):
                        b0 = sl * SLAB + blk * 128
                        f1s = fstg.tile([128, DIM], F32, tag="fstg")
                        f2s = fstg.tile([128, DIM], F32, tag="fstg")
                        nc.sync.dma_start(out=f1s[:], in_=feat1[b0:b0 + 128, :])
                        nc.sync.dma_start(out=f2s[:], in_=feat2[b0:b0 + 128, :])
                        f1b = fbfp.tile([128, DIM], BF16, tag="fbf")
                        f2b = fbfp.tile([128, DIM], BF16, tag="fbf")
                        nc.vector.tensor_copy(f1b[:], f1s[:])
                        nc.vector.tensor_copy(f2b[:], f2s[:])
                        tiles.append((f1b, f2b))
                    return tiles

                staged = stage_feat(0)
                for sl in range(NSLAB):
                    # ---- transposes from staged bf16 blocks ----
                    x1t = xtp.tile([128, 8, SLAB], BF16, tag="x1t")
                    x2t = xtp.tile([128, 8, SLAB], BF16, tag="x2t")
                    for blk in range(NSUB):
                        f1b, f2b = staged[blk]
                        transpose_in(f1b, x1t, blk)
                        transpose_in(f2b, x2t, blk)

                    # ---- Q/K/V projections (form A) ----
                    # flat layout: col = b*8 + h so a 16-sample tile is one
                    # contiguous 128-col slice (matmul APs need 1 free dim)
                    qt = qkvp.tile([128, SLAB * 8], BF16, tag="qt")
                    kt = qkvp.tile([128, SLAB * 8], BF16, tag="kt")
                    vt = qkvp.tile([128, SLAB * 8], BF16, tag="vt")
                    for wname, dst, mv, ceng in (
                        ("wq", qt, x1t, "v"), ("wk", kt, x2t, "a"), ("wv", vt, x2t, "x"),
                    ):
                        wmat = wbf[wname]
                        for oc in range(8):
                            pq = ps_work.tile([128, 512], F32, tag="work")
                            for fc in range(8):
                                nc.tensor.matmul(
                                    pq[:],
                                    wmat[:, fc, oc * 128:(oc + 1) * 128],
                                    mv[:, fc, :],
                                    start=(fc == 0), stop=(fc == 7),
                                )
                            dv = dst[:].rearrange("p (b h) -> p h b", h=8)[:, oc, :]
                            eng = ceng if ceng != "x" else ("v" if oc % 2 == 0 else "a")
                            if eng == "v":
                                nc.vector.tensor_copy(dv, pq[:])
                            else:
                                nc.scalar.copy(dv, pq[:])

                    # prefetch next slab's feat while middle runs
                    if sl + 1 < NSLAB:
                        staged = stage_feat(sl + 1)

                    # ---- middle, software-pipelined by one sub ----
                    for sub in range(NSUB):
                        sb0 = sub * SUB
                        # scores: 8 packed tiles -> 2 psum banks
                        psA = ps_sc.tile([128, 512], F32, tag="sc")
                        psB = ps_sc.tile([128, 512], F32, tag="sc")
                        for t in range(TPS):
                            o = (sb0 + t * TS) * 8
                            ps = psA if t < 4 else psB
                            nc.tensor.matmul(
                                ps[:, (t % 4) * 128:(t % 4 + 1) * 128],
                                kt[:, o:o + 128], qt[:, o:o + 128],
                                start=True, stop=True,
                            )
                        # V_pack transposes (independent of scores chain)
                        pv1 = ps_mid.tile([128, 512], BF16, tag="mid")
                        pv2 = ps_mid.tile([128, 512], BF16, tag="mid")
                        for t in range(TPS):
                            o = (sb0 + t * TS) * 8
                            pv = pv1 if t < 4 else pv2
                            nc.tensor.transpose(
                                pv[:, (t % 4) * 128:(t % 4 + 1) * 128], vt[:, o:o + 128], ident[:]
                            )
                        # stage 2 of previous sub fills PE while ACT/DVE work here
                        if pending:
                            emit_stage2(pending.pop(0))
                        # softmax pieces
                        e_sb = ep.tile([128, 1024], BF16, tag="esb")
                        nc.scalar.activation(
                            e_sb[:, 0:512], psA[:], mybir.ActivationFunctionType.Exp,
                            bias=0.0, scale=float(1.0 / SCALE),
                        )
                        nc.scalar.activation(
                            e_sb[:, 512:1024], psB[:], mybir.ActivationFunctionType.Exp,
                            bias=0.0, scale=float(1.0 / SCALE),
                        )
                        e_m = emp.tile([128, 1024], BF16, tag="em")
                        nc.vector.tensor_mul(e_m[:], e_sb[:], m01[:])
                        ps_s1 = ps_mid.tile([128, 512], F32, tag="mid")
                        ps_s2 = ps_mid.tile([128, 512], F32, tag="mid")
                        nc.tensor.matmul(ps_s1[:], ones128[:], e_m[:, 0:512], start=True, stop=True)
                        nc.tensor.matmul(ps_s2[:], ones128[:], e_m[:, 512:1024], start=True, stop=True)
                        r_sb = rp.tile([128, 1024], BF16, tag="rsb")
                        with nc.allow_low_precision(reason="softmax recip in bf16 is fine"):
                            nc.vector.reciprocal(r_sb[:, 0:512], ps_s1[:])
                            nc.vector.reciprocal(r_sb[:, 512:1024], ps_s2[:])
                        vpk_sb = vp.tile([128, 1024], BF16, tag="vpk")
                        nc.scalar.copy(vpk_sb[:, 0:512], pv1[:])
                        nc.scalar.copy(vpk_sb[:, 512:1024], pv2[:])
                        ctxt_sb = ctp.tile([128, 1024], BF16, tag="ctxt")
                        pending.append((sl, sub, e_m, r_sb, vpk_sb, ctxt_sb))

                while pending:
                    emit_stage2(pending.pop(0))

            if TIME_LOOP_N:
                with tc.For_i(0, TIME_LOOP_N, 1) as _iv:
                    emit_all()
            else:
                emit_all()
    nc.compile()
    return nc


def _numpy_fallback(feat1, feat2, Wq, bq, Wk, bk, Wv, bv, Wo, bo):
    def sm(x):
        x = x - x.max(-1, keepdims=True)
        e = np.exp(x)
        return e / e.sum(-1, keepdims=True)

    b = feat1.shape[0]
    Q = (feat1 @ Wq + bq).reshape(b, H, HD)
    K = (feat2 @ Wk + bk).reshape(b, H, HD)
    V = (feat2 @ Wv + bv).reshape(b, H, HD)
    s = np.einsum("bhd,bgd->bhg", Q, K) / SCALE
    a = sm(s)
    ctx = np.einsum("bhg,bgd->bhd", a, V).reshape(b, DIM)
    return (ctx @ Wo + bo).astype(np.float32)


def kernel(feat1, feat2, Wq, bq, Wk, bk, Wv, bv, Wo, bo):
    feat1 = np.ascontiguousarray(np.asarray(feat1, dtype=np.float32))
    feat2 = np.ascontiguousarray(np.asarray(feat2, dtype=np.float32))
    Wq = np.ascontiguousarray(np.asarray(Wq, dtype=np.float32))
    Wk = np.ascontiguousarray(np.asarray(Wk, dtype=np.float32))
    Wv = np.ascontiguousarray(np.asarray(Wv, dtype=np.float32))
    Wo = np.ascontiguousarray(np.asarray(Wo, dtype=np.float32))
    bq, bk, bv, bo = (np.asarray(x, dtype=np.float32) for x in (bq, bk, bv, bo))
    if any(np.abs(x).max() > 0 for x in (bq, bk, bv, bo) if x.size):
        return _numpy_fallback(feat1, feat2, Wq, bq, Wk, bk, Wv, bv, Wo, bo)

    from concourse.bass_utils import run_bass_kernel_spmd

    if "nc" not in _nc_cache:
        _nc_cache["nc"] = build_nc()
    nc = _nc_cache["nc"]

    in_maps = []
    for c in range(NCORES):
        sl = slice(c * B_LOC, (c + 1) * B_LOC)
        in_maps.append({
            "feat1": feat1[sl], "feat2": feat2[sl],
            "Wq": Wq, "Wk": Wk, "Wv": Wv, "Wo": Wo,
        })
    res = run_bass_kernel_spmd(nc, in_maps, list(range(NCORES)))
    return np.concatenate([res.results[c]["out"] for c in range(NCORES)], axis=0)


# revision 4
# speedup vs baseline: 22.5563x; 1.8914x over previous
"""CrossAttentionFusion TRN2 kernel v2: fused single-pass bf16 pipeline, 8-core data parallel.

Per core (B_loc = 2048), per slab of 512 samples:
  - DMA feat blocks, DVE-convert fp32->bf16, PE-transpose -> x1t/x2t [f, b] bf16
  - Q^T/K^T/V^T via form-A matmuls (stationary = W bf16 chunks): qt/kt/vt [d, h, b]
  - middle per 16-sample tile: ONE packed 128x128 scores matmul (cols (s,h), rows (s,g)),
    exp via ACT, block-diag mask via DVE, denominators via ones-matmul (replicated to all
    rows), V^T tile -> V_pack via PE transpose, UNNORMALIZED ctx^T = V_pack^T @ e_m,
    then normalize folded into the PSUM->SBUF copy (DVE mul by recip of sums)
  - out = ctx @ Wo via form-B (stationary = ctx^T chunks straight from SBUF)
No DRAM intermediates; everything bf16 except PSUM accumulation + softmax stats.
"""

import sys

sys.path.insert(0, "/opt/trn_rl_repo")

import numpy as np
import concourse.bacc as bacc
import concourse.mybir as mybir
import concourse.tile as tile
from concourse.masks import make_identity, make_block_diagonal

B = 16384
DIM = 1024
H = 8
HD = 128
SCALE = float(np.sqrt(HD))
NCORES = 8
B_LOC = B // NCORES  # 2048
SLAB = 512
NSLAB = B_LOC // SLAB  # 4
SUB = 128
NSUB = SLAB // SUB  # 4
TS = 16  # samples per packed tile
TPS = SUB // TS  # 8 tiles per sub

F32 = mybir.dt.float32
BF16 = mybir.dt.bfloat16

_nc_cache = {}
TIME_LOOP_N = None


def build_nc():
    nc = bacc.Bacc(None)
    feat1 = nc.declare_dram_parameter("feat1", [B_LOC, DIM], F32, isOutput=False)
    feat2 = nc.declare_dram_parameter("feat2", [B_LOC, DIM], F32, isOutput=False)
    Wq = nc.declare_dram_parameter("Wq", [DIM, DIM], F32, isOutput=False)
    Wk = nc.declare_dram_parameter("Wk", [DIM, DIM], F32, isOutput=False)
    Wv = nc.declare_dram_parameter("Wv", [DIM, DIM], F32, isOutput=False)
    Wo = nc.declare_dram_parameter("Wo", [DIM, DIM], F32, isOutput=False)
    out = nc.declare_dram_parameter("out", [B_LOC, DIM], F32, isOutput=True)

    w_view = lambda W: W.rearrange("(c p) n -> p c n", p=128)  # f=(c,p), out n

    with tile.TileContext(nc) as tc:
        with (
            tc.tile_pool(name="const", bufs=1) as cpool,
            tc.tile_pool(name="wbf", bufs=1) as wpool,
            tc.tile_pool(name="wstage", bufs=2) as wstg,
            tc.tile_pool(name="fstage", bufs=4) as fstg,
            tc.tile_pool(name="fbf", bufs=4) as fbfp,
            tc.tile_pool(name="xt", bufs=1) as xtp,
            tc.tile_pool(name="qkv", bufs=1) as qkvp,
            tc.tile_pool(name="esb", bufs=2) as ep,
            tc.tile_pool(name="em", bufs=2) as emp,
            tc.tile_pool(name="rsb", bufs=2) as rp,
            tc.tile_pool(name="attn", bufs=2) as ap,
            tc.tile_pool(name="vpk", bufs=2) as vp,
            tc.tile_pool(name="ctxt", bufs=2) as ctp,
            tc.tile_pool(name="outsb", bufs=4) as op,
            tc.tile_pool(name="ps_work", bufs=2, space="PSUM") as ps_work,
            tc.tile_pool(name="ps_sc", bufs=2, space="PSUM") as ps_sc,
            tc.tile_pool(name="ps_mid", bufs=4, space="PSUM") as ps_mid,
        ):
            ident = cpool.tile([128, 128], BF16, tag="ident")
            make_identity(nc, ident)
            # m01[128, 1024]: block t (128 cols) has blockdiag8 ones pattern:
            # m01[8s+g, t*128 + 8s'+h] = (s == s')
            m01 = cpool.tile([128, 1024], BF16, tag="m01")
            make_block_diagonal(nc, m01[:, 0:128], 8)
            for t in range(1, 8):
                nc.vector.tensor_copy(m01[:, t * 128:(t + 1) * 128], m01[:, 0:128])
            ones128 = cpool.tile([128, 128], BF16, tag="ones128")
            nc.gpsimd.memset(ones128[:], 1.0)

            def load_weights():
                """DMA fp32 weights, convert to bf16 in SBUF. Returns dict."""
                wbf = {}
                for i, (name, W) in enumerate((("wq", Wq), ("wk", Wk), ("wv", Wv), ("wo", Wo))):
                    wbf[name] = wpool.tile([128, 8, 1024], BF16, tag=name, name=name)
                for i, (name, W) in enumerate((("wq", Wq), ("wk", Wk), ("wv", Wv), ("wo", Wo))):
                    for j in range(4):  # 2 chunks per step
                        stg = wstg.tile([128, 2, 1024], F32, tag="wstage")
                        nc.gpsimd.dma_start(out=stg[:], in_=w_view(W)[:, 2 * j:2 * j + 2, :])
                        if (i * 4 + j) % 2 == 0:
                            nc.vector.tensor_copy(wbf[name][:, 2 * j:2 * j + 2, :], stg[:])
                        else:
                            nc.scalar.copy(wbf[name][:, 2 * j:2 * j + 2, :], stg[:])
                return wbf

            def transpose_in(fb, dst, blk):
                """fb [128,1024] bf16 -> dst[:, :, blk*128:(blk+1)*128] (dst [128,8,512])."""
                for t in range(2):
                    pt = ps_work.tile([128, 512], BF16, tag="work")
                    for j in range(4):
                        c = 4 * t + j
                        nc.tensor.transpose(
                            pt[:, j * 128:(j + 1) * 128],
                            fb[:, c * 128:(c + 1) * 128],
                            ident[:],
                        )
                    nc.scalar.copy(
                        dst[:, 4 * t:4 * t + 4, blk * 128:(blk + 1) * 128],
                        pt[:].rearrange("p (j b) -> p j b", j=4),
                    )

            def emit_all():
                wbf = load_weights()

                # pipeline state: pending sub whose ctx+O aren't emitted yet
                pending = []

                def emit_stage2(st):
                    """ctx matmuls (unnormalized) + normalize + O-projection for a finished sub."""
                    sl, sub, e_m, r_sb, vpk_sb, ctxt_sb = st
                    pc1 = ps_mid.tile([128, 512], F32, tag="mid")
                    pc2 = ps_mid.tile([128, 512], F32, tag="mid")
                    for t in range(TPS):
                        pc = pc1 if t < 4 else pc2
                        nc.tensor.matmul(
                            pc[:, (t % 4) * 128:(t % 4 + 1) * 128],
                            vpk_sb[:, t * 128:(t + 1) * 128],
                            e_m[:, t * 128:(t + 1) * 128],
                            start=True, stop=True,
                        )
                    # ctxt col = h*128 + t*16 + s so O-proj stationary is contiguous
                    ctxt_v = ctxt_sb[:].rearrange("d (h t s) -> d t h s", h=8, t=8, s=16)
                    with nc.allow_low_precision(reason="softmax normalize in bf16"):
                        for pi, pc in ((0, pc1), (1, pc2)):
                            nc.vector.tensor_mul(
                                ctxt_v[:, 4 * pi:4 * pi + 4, :, :],
                                pc[:].rearrange("p (t s h) -> p t h s", t=4, s=16, h=8),
                                r_sb[:, 512 * pi:512 * (pi + 1)].rearrange(
                                    "p (t s h) -> p t h s", t=4, s=16, h=8),
                            )
                    # O-projection for this sub: out rows = b = sub block
                    b0 = sl * SLAB + sub * SUB
                    for half in range(2):
                        po = ps_work.tile([128, 512], F32, tag="work")
                        for h in range(8):
                            nc.tensor.matmul(
                                po[:],
                                ctxt_sb[:, h * 128:(h + 1) * 128],
                                wbf["wo"][:, h, half * 512:(half + 1) * 512],
                                start=(h == 0), stop=(h == 7),
                            )
                        osb = op.tile([128, 512], F32, tag="osb")
                        if half == 0:
                            nc.vector.tensor_copy(osb[:], po[:])
                        else:
                            nc.scalar.copy(osb[:], po[:])
                        nc.sync.dma_start(
                            out=out[b0:b0 + 128, half * 512:(half + 1) * 512],
                            in_=osb[:],
                        )

                def stage_feat(sl):
                    """DMA + bf16-convert one slab's feat blocks; returns list of bf16 tiles."""
                    tiles = []
                    for blk in range(NSUB):
                        b0 = sl * SLAB + blk * 128
                        f1s = fstg.tile([128, DIM], F32, tag="fstg")
                        f2s = fstg.tile([128, DIM], F32, tag="fstg")
                        nc.sync.dma_start(out=f1s[:], in_=feat1[b0:b0 + 128, :])
                        nc.sync.dma_start(out=f2s[:], in_=feat2[b0:b0 + 128, :])
                        f1b = fbfp.tile([128, DIM], BF16, tag="fbf")
                        f2b = fbfp.tile([128, DIM], BF16, tag="fbf")
                        nc.vector.tensor_copy(f1b[:], f1s[:])
                        nc.vector.tensor_copy(f2b[:], f2s[:])
                        tiles.append((f1b, f2b))
                    return tiles

                staged = stage_feat(0)
                for sl in range(NSLAB):
                    # ---- transposes from staged bf16 blocks ----
                    x1t = xtp.tile([128, 8, SLAB], BF16, tag="x1t")
                    x2t = xtp.tile([128, 8, SLAB], BF16, tag="x2t")
                    for blk in range(NSUB):
                        f1b, f2b = staged[blk]
                        transpose_in(f1b, x1t, blk)
                        transpose_in(f2b, x2t, blk)

                    # ---- Q/K/V projections (form A) ----
                    # flat layout: col = b*8 + h so a 16-sample tile is one
                    # contiguous 128-col slice (matmul APs need 1 free dim)
                    qt = qkvp.tile([128, SLAB * 8], BF16, tag="qt")
                    kt = qkvp.tile([128, SLAB * 8], BF16, tag="kt")
                    vt = qkvp.tile([128, SLAB * 8], BF16, tag="vt")
                    for wname, dst, mv, ceng in (
                        ("wq", qt, x1t, "v"), ("wk", kt, x2t, "a"), ("wv", vt, x2t, "x"),
                    ):
                        wmat = wbf[wname]
                        for oc in range(8):
                            pq = ps_work.tile([128, 512], F32, tag="work")
                            for fc in range(8):
                                nc.tensor.matmul(
                                    pq[:],
                                    wmat[:, fc, oc * 128:(oc + 1) * 128],
                                    mv[:, fc, :],
                                    start=(fc == 0), stop=(fc == 7),
                                )
                            dv = dst[:].rearrange("p (b h) -> p h b", h=8)[:, oc, :]
                            eng = ceng if ceng != "x" else ("v" if oc % 2 == 0 else "a")
                            if eng == "v":
                                nc.vector.tensor_copy(dv, pq[:])
                            else:
                                nc.scalar.copy(dv, pq[:])

                    # prefetch next slab's feat while middle runs
                    if sl + 1 < NSLAB:
                        staged = stage_feat(sl + 1)

                    # ---- middle, software-pipelined by one sub ----
                    for sub in range(NSUB):
                        sb0 = sub * SUB
                        # scores: 8 packed tiles -> 2 psum banks
                        psA = ps_sc.tile([128, 512], F32, tag="sc")
                        psB = ps_sc.tile([128, 512], F32, tag="sc")
                        for t in range(TPS):
                            o = (sb0 + t * TS) * 8
                            ps = psA if t < 4 else psB
                            nc.tensor.matmul(
                                ps[:, (t % 4) * 128:(t % 4 + 1) * 128],
                                kt[:, o:o + 128], qt[:, o:o + 128],
                                start=True, stop=True,
                            )
                        # V_pack transposes (independent of scores chain)
                        pv1 = ps_mid.tile([128, 512], BF16, tag="mid")
                        pv2 = ps_mid.tile([128, 512], BF16, tag="mid")
                        for t in range(TPS):
                            o = (sb0 + t * TS) * 8
                            pv = pv1 if t < 4 else pv2
                            nc.tensor.transpose(
                                pv[:, (t % 4) * 128:(t % 4 + 1) * 128], vt[:, o:o + 128], ident[:]
                            )
                        # stage 2 of previous sub fills PE while ACT/DVE work here
                        if pending:
                            emit_stage2(pending.pop(0))
                        # softmax pieces
                        e_sb = ep.tile([128, 1024], BF16, tag="esb")
                        nc.scalar.activation(
                            e_sb[:, 0:512], psA[:], mybir.ActivationFunctionType.Exp,
                            bias=0.0, scale=float(1.0 / SCALE),
                        )
                        nc.scalar.activation(
                            e_sb[:, 512:1024], psB[:], mybir.ActivationFunctionType.Exp,
                            bias=0.0, scale=float(1.0 / SCALE),
                        )
                        e_m = emp.tile([128, 1024], BF16, tag="em")
                        nc.vector.tensor_mul(e_m[:], e_sb[:], m01[:])
                        ps_s1 = ps_mid.tile([128, 512], F32, tag="mid")
                        ps_s2 = ps_mid.tile([128, 512], F32, tag="mid")
                        nc.tensor.matmul(ps_s1[:], ones128[:], e_m[:, 0:512], start=True, stop=True)
                        nc.tensor.matmul(ps_s2[:], ones128[:], e_m[:, 512:1024], start=True, stop=True)
                        r_sb = rp.tile([128, 1024], BF16, tag="rsb")
                        with nc.allow_low_precision(reason="softmax recip in bf16 is fine"):
                            nc.vector.reciprocal(r_sb[:, 0:512], ps_s1[:])
                            nc.vector.reciprocal(r_sb[:, 512:1024], ps_s2[:])
                        vpk_sb = vp.tile([128, 1024], BF16, tag="vpk")
                        nc.scalar.copy(vpk_sb[:, 0:512], pv1[:])
                        nc.scalar.copy(vpk_sb[:, 512:1024], pv2[:])
                        ctxt_sb = ctp.tile([128, 1024], BF16, tag="ctxt")
                        pending.append((sl, sub, e_m, r_sb, vpk_sb, ctxt_sb))

                while pending:
                    emit_stage2(pending.pop(0))

            if TIME_LOOP_N:
                with tc.For_i(0, TIME_LOOP_N, 1) as _iv:
                    emit_all()
            else:
                emit_all()
    nc.compile()
    return nc


def _numpy_fallback(feat1, feat2, Wq, bq, Wk, bk, Wv, bv, Wo, bo):
    def sm(x):
        x = x - x.max(-1, keepdims=True)
        e = np.exp(x)
        return e / e.sum(-1, keepdims=True)

    b = feat1.shape[0]
    Q = (feat1 @ Wq + bq).reshape(b, H, HD)
    K = (feat2 @ Wk + bk).reshape(b, H, HD)
    V = (feat2 @ Wv + bv).reshape(b, H, HD)
    s = np.einsum("bhd,bgd->bhg", Q, K) / SCALE
    a = sm(s)
    ctx = np.einsum("bhg,bgd->bhd", a, V).reshape(b, DIM)
    return (ctx @ Wo + bo).astype(np.float32)


def kernel(feat1, feat2, Wq, bq, Wk, bk, Wv, bv, Wo, bo):
    feat1 = np.ascontiguousarray(np.asarray(feat1, dtype=np.float32))
    feat2 = np.ascontiguousarray(np.asarray(feat2, dtype=np.float32))
    Wq = np.ascontiguousarray(np.asarray(Wq, dtype=np.float32))
    Wk = np.ascontiguousarray(np.asarray(Wk, dtype=np.float32))
    Wv = np.ascontiguousarray(np.asarray(Wv, dtype=np.float32))
    Wo = np.ascontiguousarray(np.asarray(Wo, dtype=np.float32))
    bq, bk, bv, bo = (np.asarray(x, dtype=np.float32) for x in (bq, bk, bv, bo))
    if any(np.abs(x).max() > 0 for x in (bq, bk, bv, bo) if x.size):
        return _numpy_fallback(feat1, feat2, Wq, bq, Wk, bk, Wv, bv, Wo, bo)

    from concourse.bass_utils import run_bass_kernel_spmd

    if "nc" not in _nc_cache:
        _nc_cache["nc"] = build_nc()
    nc = _nc_cache["nc"]

    in_maps = []
    for c in range(NCORES):
        sl = slice(c * B_LOC, (c + 1) * B_LOC)
        in_maps.append({
            "feat1": feat1[sl], "feat2": feat2[sl],
            "Wq": Wq, "Wk": Wk, "Wv": Wv, "Wo": Wo,
        })
    res = run_bass_kernel_spmd(nc, in_maps, list(range(NCORES)))
    return np.concatenate([res.results[c]["out"] for c in range(NCORES)], axis=0)
